# revision 12
# baseline (speedup 1.0000x reference)
"""Trainium2 Bass kernel for nn_Attention_54855322304634 (8 NeuronCores).

Strategy (sequence-parallel + per-batch K/V AllGather):
- rows = flattened (B*S) = 4096; core c owns rows [512c, 512c+512); batch = c//4.
- AdaLN modulation: each core computes a 768-wide chunk (index c%4) of
  mod = silu(ada_b) @ mod_w.T + mod_b for ITS batch; AllGather over the
  4 cores of the batch reassembles the full [3072] modulation vector.
- LayerNorm + modulation in [rows, H] layout, PE-transpose to h_T [H, rows].
- QKV projection with host-transposed w_qkv (f32r matmuls, full PE rate).
- Per-head QK LayerNorm + RoPE in [rows, heads, hd]; q folded with 1/sqrt(hd);
  PE-transpose q,k to [2*64 pairdim, headpair, rows] bf16.
- AllGather (per-batch groups) of bf16 k_T and v.
- Attention per head: scores_T = K_hT.T @ q_hT (softmax axis on partitions),
  exp with NO max subtraction (max |score| ~18 for this problem's data),
  PV matmul with ones-augmented V so the denominator falls out of the same
  accumulation; normalize via reciprocal + stride-0 DMA partition broadcast.
- o-proj with host-transposed w_o (f32r), gate multiply, write own row-slice.
"""

import sys

if "/opt/trn_rl_repo" not in sys.path:
    sys.path.insert(0, "/opt/trn_rl_repo")

import numpy as np

import concourse.bass as bass
import concourse.tile as tile
from concourse import bacc, mybir
from concourse.bass_utils import run_bass_kernel_spmd
from concourse.masks import make_identity

F32 = mybir.dt.float32
F32R = mybir.dt.float32r
BF16 = mybir.dt.bfloat16
AX = mybir.AxisListType
OP = mybir.AluOpType
ACT = mybir.ActivationFunctionType

NH, HD, H, B, S, A = 16, 64, 1024, 2, 2048, 1024
EPS = 1e-5
R = B * S                # 4096 rows total
RC = R // 8              # 512 rows per core
NRT = RC // 128          # 4 row tiles per core
KC = S // 128            # 16 key chunks per head
H3 = 3 * H

_KF = 128 * 8 * RC       # flat bf16 elems of k_T region in the kv AG buffer
_VF = RC * H             # flat bf16 elems of v region


def _bc(ap, p):
    """Stride-0 partition broadcast to [p, ...].

    If dim0 has size 1 it is replaced; otherwise a new stride-0 partition
    dim is prepended (AP dims are [stride, size] pairs).
    """
    dims = list(ap.ap)
    if dims[0][1] == 1:
        dims = dims[1:]
    return bass.AP(tensor=ap.tensor, offset=ap.offset, ap=[[0, p]] + dims)


def _r(ap):
    return ap.bitcast(F32R)


def _emit(tc, ins, out):
    nc = tc.nc
    x_in, freqs_in, wqkvT_in, woT_in, modwT_in, modb_in, ada_in, lnw_in, qnw_in, knw_in = (
        ins["x"], ins["freqs"], ins["wqkvT"], ins["woT"], ins["modwT"],
        ins["modb"], ins["ada"], ins["lnw"], ins["qnw"], ins["knw"],
    )

    const = tc.alloc_tile_pool(name="const", bufs=1)
    pers = tc.alloc_tile_pool(name="pers", bufs=1)
    dram = tc.alloc_tile_pool(name="dram", bufs=1, space="DRAM")

    # ---------------- constants ----------------
    ident = const.tile([128, 128], F32)
    make_identity(nc, ident)
    eps128 = const.tile([128, 1], F32)
    nc.vector.memset(eps128, EPS)

    lnw_rep = const.tile([128, H], F32)
    nc.sync.dma_start(out=lnw_rep, in_=_bc(lnw_in, 128))
    qn_eff = const.tile([128, HD], F32)
    nc.sync.dma_start(out=qn_eff, in_=_bc(qnw_in, 128))
    nc.vector.tensor_scalar_mul(qn_eff, qn_eff, 0.125)  # fold 1/sqrt(hd) into q
    kn_rep = const.tile([128, HD], F32)
    nc.sync.dma_start(out=kn_rep, in_=_bc(knw_in, 128))

    f0t, f1t = [], []
    for rt in range(NRT):
        f0 = const.tile([128, 32], F32, tag=f"f0_{rt}")
        f1 = const.tile([128, 32], F32, tag=f"f1_{rt}")
        nc.sync.dma_start(out=f0, in_=freqs_in[rt * 128:(rt + 1) * 128, 0, :])
        nc.sync.dma_start(out=f1, in_=freqs_in[rt * 128:(rt + 1) * 128, 1, :])
        f0t.append(f0)
        f1t.append(f1)

    # ---------------- modulation chunk + AllGather ----------------
    ag1_in = dram.tile([1, 768], F32)
    ag1_out = dram.tile([4, 768], F32)

    with tc.tile_pool(name="modp", bufs=1) as modp, \
         tc.tile_pool(name="modpsum", bufs=2, space="PSUM") as modpsum:
        modwT_sb = modp.tile([128, 8, 768], F32R)
        nc.sync.dma_start(out=modwT_sb, in_=modwT_in.rearrange("(kt p) m -> p kt m", p=128).bitcast(F32R))
        modb_sb = modp.tile([128, 6], F32)
        nc.sync.dma_start(out=modb_sb, in_=modb_in)
        ada_sb = modp.tile([128, 8], F32)
        nc.sync.dma_start(out=ada_sb, in_=ada_in)
        # fp32r matmul needs even innermost free sizes -> pad N=1 to N=2
        silu_sb = modp.tile([128, 8, 2], F32R)
        nc.vector.memset(silu_sb[:].bitcast(F32), 0.0)
        nc.scalar.activation(out=silu_sb[:, :, 0], in_=ada_sb, func=ACT.Sigmoid)
        nc.vector.tensor_tensor(
            out=silu_sb[:, :, 0], in0=silu_sb[:, :, 0], in1=ada_sb, op=OP.mult)

        mod_sb = modp.tile([128, 6], F32)
        for t in range(6):
            ps = modpsum.tile([128, 2], F32, tag="modps")
            for kt in range(8):
                nc.tensor.matmul(
                    ps, modwT_sb[:, kt, t * 128:(t + 1) * 128],
                    silu_sb[:, kt, :], start=(kt == 0), stop=(kt == 7),
                )
            nc.vector.tensor_tensor(
                out=mod_sb[:, t:t + 1], in0=ps[:, 0:1], in1=modb_sb[:, t:t + 1], op=OP.add)
        with nc.allow_non_contiguous_dma(reason="3KB mod chunk scatter"):
            nc.sync.dma_start(
                out=ag1_in[0, :].rearrange("(t p) -> p t", p=128), in_=mod_sb)

    nc.gpsimd.collective_compute(
        "AllGather", OP.bypass,
        ins=[ag1_in[:].opt()], outs=[ag1_out[:].opt()],
        replica_groups=[[0, 1, 2, 3], [4, 5, 6, 7]],
    )

    # vec3_rep[p, :] = full [3072] modulation vector, replicated on partitions
    vec3_rep = pers.tile([128, 3072], F32)
    nc.sync.dma_start(out=vec3_rep, in_=_bc(ag1_out[:].rearrange("a b -> (a b)"), 128))
    scale1p = pers.tile([128, H], F32)
    nc.vector.tensor_scalar_add(scale1p, vec3_rep[:, 0:H], 1.0)
    nc.vector.tensor_tensor(out=scale1p, in0=scale1p, in1=lnw_rep, op=OP.mult)
    shift_v = vec3_rep[:, H:2 * H]
    gate_v = vec3_rep[:, 2 * H:3 * H]

    # ---------------- phase A: LN + modulate + transpose to h_T ----------------
    hT = pers.tile([128, 8, RC], F32R)   # [H-part, H-chunk, rows]

    with tc.tile_pool(name="xa", bufs=2) as xa, \
         tc.tile_pool(name="stats", bufs=2) as stats, \
         tc.tile_pool(name="tpsum", bufs=4, space="PSUM") as tpsum:
        for rt in range(NRT):
            xt = xa.tile([128, H], F32, tag="xt")
            nc.sync.dma_start(out=xt, in_=x_in[rt * 128:(rt + 1) * 128, :])
            st = stats.tile([128, 2, 6], F32, tag="bnst")
            nc.vector.bn_stats(out=st[:, 0, :], in_=xt[:, 0:512])
            nc.vector.bn_stats(out=st[:, 1, :], in_=xt[:, 512:1024])
            mv = stats.tile([128, 2], F32, tag="bnmv")
            nc.vector.bn_aggr(out=mv, in_=st)
            rstd = stats.tile([128, 1], F32, tag="rstd")
            nc.scalar.activation(out=rstd, in_=mv[:, 1:2], func=ACT.Sqrt, bias=eps128)
            nc.vector.reciprocal(rstd, rstd)
            hmod = xa.tile([128, H], F32, tag="hmod")
            nc.vector.tensor_scalar(
                out=hmod, in0=xt, scalar1=mv[:, 0:1], scalar2=rstd,
                op0=OP.subtract, op1=OP.mult)
            nc.vector.tensor_tensor(out=hmod, in0=hmod, in1=scale1p, op=OP.mult)
            nc.vector.tensor_tensor(out=hmod, in0=hmod, in1=shift_v, op=OP.add)
            for kt in range(8):
                pt = tpsum.tile([128, 128], F32, tag="tp")
                nc.tensor.transpose(pt, hmod[:, kt * 128:(kt + 1) * 128], ident)
                nc.vector.tensor_copy(out=hT[:, kt, rt * 128:(rt + 1) * 128], in_=pt)

    # ---------------- phase B: QKV + qk-LN + RoPE + transposes ----------------
    qT = pers.tile([128, 8, RC], BF16)
    ag2_in = dram.tile([1, _KF + _VF], BF16)
    ag2_out = dram.tile([4, _KF + _VF], BF16)
    k_region = ag2_in[0, 0:_KF].rearrange("(p h r) -> p h r", p=128, h=8)
    v_region = ag2_in[0, _KF:].rearrange("(r c) -> r c", r=RC)

    def qk_process(nc_, tmp, rt, is_q, dst, hp0):
        """tmp: [128, 512] f32 = 8 heads of q or k for row tile rt."""
        t3 = tmp[:].rearrange("p (h d) -> p h d", h=8)
        s1 = stats2.tile([128, 8], F32, tag="s1")
        s2 = stats2.tile([128, 8], F32, tag="s2")
        sq = work.tile([128, 512], F32, tag="sq")
        nc_.vector.tensor_reduce(out=s1, in_=t3, axis=AX.X, op=OP.add)
        nc_.scalar.activation(out=sq, in_=tmp[:], func=ACT.Square)
        nc_.vector.tensor_reduce(out=s2, in_=sq[:].rearrange("p (h d) -> p h d", h=8),
                                 axis=AX.X, op=OP.add)
        nc_.vector.tensor_scalar_mul(s1, s1, 1.0 / HD)           # mean
        nc_.vector.tensor_scalar_mul(s2, s2, 1.0 / HD)           # mean(x^2)
        m2 = stats2.tile([128, 8], F32, tag="m2")
        nc_.vector.tensor_tensor(out=m2, in0=s1, in1=s1, op=OP.mult)
        nc_.vector.tensor_tensor(out=s2, in0=s2, in1=m2, op=OP.subtract)  # var
        nc_.scalar.activation(out=s2, in_=s2, func=ACT.Sqrt, bias=eps128)
        nc_.vector.reciprocal(s2, s2)                            # rstd
        y = work.tile([128, 512], F32, tag="y")
        y3 = y[:].rearrange("p (h d) -> p h d", h=8)
        nc_.vector.tensor_tensor(out=y3, in0=t3, in1=s1[:, :, None].to_broadcast((128, 8, 64)), op=OP.subtract)
        nc_.vector.tensor_tensor(out=y3, in0=y3, in1=s2[:, :, None].to_broadcast((128, 8, 64)), op=OP.mult)
        w_rep = qn_eff if is_q else kn_rep
        nc_.vector.tensor_tensor(out=y3, in0=y3, in1=w_rep[:, None, :].to_broadcast((128, 8, 64)), op=OP.mult)
        # RoPE
        y4 = y[:].rearrange("p (h d2 two) -> p h d2 two", h=8, two=2)
        ro = work.tile([128, 512], F32, tag="ro")
        ro4 = ro[:].rearrange("p (h d2 two) -> p h d2 two", h=8, two=2)
        tm = work.tile([128, 256], F32, tag="tm")
        tm3 = tm[:].rearrange("p (h d2) -> p h d2", h=8)
        f0b = f0t[rt][:, None, :].to_broadcast((128, 8, 32))
        f1b = f1t[rt][:, None, :].to_broadcast((128, 8, 32))
        nc_.vector.tensor_tensor(out=ro4[:, :, :, 0], in0=y4[:, :, :, 0], in1=f0b, op=OP.mult)
        nc_.vector.tensor_tensor(out=tm3, in0=y4[:, :, :, 1], in1=f1b, op=OP.mult)
        nc_.vector.tensor_tensor(out=ro4[:, :, :, 0], in0=ro4[:, :, :, 0], in1=tm3, op=OP.subtract)
        nc_.vector.tensor_tensor(out=ro4[:, :, :, 1], in0=y4[:, :, :, 1], in1=f0b, op=OP.mult)
        nc_.vector.tensor_tensor(out=tm3, in0=y4[:, :, :, 0], in1=f1b, op=OP.mult)
        nc_.vector.tensor_tensor(out=ro4[:, :, :, 1], in0=ro4[:, :, :, 1], in1=tm3, op=OP.add)
        # transpose 4 head-pairs -> dst[:, hp0+pair, rt*128:...]
        for pr in range(4):
            pt = tpsum2.tile([128, 128], F32, tag="tp2")
            nc_.tensor.transpose(pt, ro[:, pr * 128:(pr + 1) * 128], ident)
            nc_.vector.tensor_copy(
                out=dst[:, hp0 + pr, rt * 128:(rt + 1) * 128], in_=pt)

    with tc.tile_pool(name="wq", bufs=16) as wq, \
         tc.tile_pool(name="work", bufs=2) as work, \
         tc.tile_pool(name="stats2", bufs=2) as stats2, \
         tc.tile_pool(name="kstage", bufs=2) as kstage, \
         tc.tile_pool(name="qkpsum", bufs=2, space="PSUM") as qkpsum, \
         tc.tile_pool(name="tpsum2", bufs=2, space="PSUM") as tpsum2:
        # k chunks first (2,3), then v (4,5), then q (0,1) so the AG can fire early
        for nch in (2, 3, 4, 5, 0, 1):
            wts = []
            for kt in range(8):
                wt = wq.tile([128, 512], F32R, tag="wt")
                nc.sync.dma_start(
                    out=wt,
                    in_=wqkvT_in[kt * 128:(kt + 1) * 128, nch * 512:(nch + 1) * 512].bitcast(F32R))
                wts.append(wt)
            kstg = None
            if nch in (2, 3):
                kstg = kstage.tile([128, 4, 512], BF16, tag="kstg")
            for rt in range(NRT):
                ps = qkpsum.tile([128, 512], F32, tag="qkps")
                for kt in range(8):
                    nc.tensor.matmul(
                        ps, hT[:, kt, rt * 128:(rt + 1) * 128], wts[kt],
                        start=(kt == 0), stop=(kt == 7))
                if nch >= 4:      # v chunk: cast + ship to AG buffer
                    vt = work.tile([128, 512], BF16, tag="vt")
                    nc.vector.tensor_copy(out=vt, in_=ps)
                    nc.sync.dma_start(
                        out=v_region[rt * 128:(rt + 1) * 128,
                                     (nch - 4) * 512:(nch - 3) * 512],
                        in_=vt)
                else:
                    tmp = work.tile([128, 512], F32, tag="tmp")
                    nc.vector.tensor_copy(out=tmp, in_=ps)
                    if nch in (0, 1):   # q
                        qk_process(nc, tmp, rt, True, qT, nch * 4)
                    else:               # k -> staging then AG buffer
                        qk_process(nc, tmp, rt, False, kstg, 0)
            if nch in (2, 3):
                nc.sync.dma_start(
                    out=k_region[:, (nch - 2) * 4:(nch - 1) * 4, :], in_=kstg)

    nc.gpsimd.collective_compute(
        "AllGather", OP.bypass,
        ins=[ag2_in[:].opt()], outs=[ag2_out[:].opt()],
        replica_groups=[[0, 1, 2, 3], [4, 5, 6, 7]],
    )

    # ---------------- phase C: attention ----------------
    woT_sb = pers.tile([128, 8, H], F32R)
    nc.sync.dma_start(out=woT_sb, in_=woT_in.rearrange("(kt p) m -> p kt m", p=128).bitcast(F32R))

    KT = pers.tile([128, 8, S], BF16)
    for r in range(4):
        nc.sync.dma_start(
            out=KT[:, :, r * RC:(r + 1) * RC],
            in_=ag2_out[r, 0:_KF].rearrange("(p h q) -> p h q", p=128, h=8))

    oT = pers.tile([128, 8, RC], F32R)

    with tc.tile_pool(name="vh", bufs=3) as vhp, \
         tc.tile_pool(name="esb", bufs=4) as esb, \
         tc.tile_pool(name="rec", bufs=2) as recp, \
         tc.tile_pool(name="spsum", bufs=4, space="PSUM") as spsum, \
         tc.tile_pool(name="opsum", bufs=2, space="PSUM") as opsum:
        for h in range(NH):
            hp, lo = h // 2, (h % 2) * 64
            vh = vhp.tile([128, KC, 65], BF16, tag="vh")
            for r in range(4):
                nc.sync.dma_start(
                    out=vh[:, r * 4:(r + 1) * 4, 0:64],
                    in_=ag2_out[r, _KF:].rearrange(
                        "(a p c) -> p a c", a=4, p=128)[:, :, h * 64:(h + 1) * 64])
            nc.vector.memset(vh[:, :, 64:65], 1.0)
            q_h = qT[lo:lo + 64, hp, :]
            po = opsum.tile([128, 512], F32, tag="ops")
            for kc in range(KC):
                ps = spsum.tile([128, 512], F32, tag="sps")
                nc.tensor.matmul(
                    ps, KT[lo:lo + 64, hp, kc * 128:(kc + 1) * 128], q_h,
                    start=True, stop=True)
                et = esb.tile([128, 512], BF16, tag="et")
                nc.scalar.activation(out=et, in_=ps, func=ACT.Exp)
                nc.tensor.matmul(
                    po[0:65, :], vh[:, kc, :], et,
                    start=(kc == 0), stop=(kc == KC - 1))
            rec = recp.tile([128, 512], F32, tag="rec")
            nc.vector.reciprocal(rec[64:65, :], po[64:65, :])
            dden = dram.tile([1, 512], F32, tag=f"dden{h % 4}")
            nc.sync.dma_start(out=dden, in_=rec[64:65, :])
            recb = recp.tile([64, 512], F32, tag="recb")
            nc.sync.dma_start(out=recb, in_=_bc(dden[:], 64))
            o_n = recp.tile([64, 512], F32R, tag="o_n")
            nc.vector.tensor_tensor(out=o_n, in0=po[0:64, :], in1=recb, op=OP.mult)
            nc.sync.dma_start(out=oT[lo:lo + 64, hp, :], in_=o_n)

    # ---------------- phase D: o-proj + gate ----------------
    with tc.tile_pool(name="outp", bufs=2) as outp, \
         tc.tile_pool(name="prpsum", bufs=2, space="PSUM") as prpsum:
        for rt in range(NRT):
            for nch in range(2):
                ps = prpsum.tile([128, 512], F32, tag="prps")
                for kt in range(8):
                    nc.tensor.matmul(
                        ps, oT[:, kt, rt * 128:(rt + 1) * 128],
                        woT_sb[:, kt, nch * 512:(nch + 1) * 512],
                        start=(kt == 0), stop=(kt == 7))
                ot = outp.tile([128, 512], F32, tag="ot")
                nc.vector.tensor_tensor(
                    out=ot, in0=ps, in1=gate_v[:, nch * 512:(nch + 1) * 512],
                    op=OP.mult)
                nc.sync.dma_start(
                    out=out[rt * 128:(rt + 1) * 128, nch * 512:(nch + 1) * 512],
                    in_=ot)

    dram.release()
    pers.release()
    const.release()


_CACHE = {}


def _build():
    if "nc" in _CACHE:
        return _CACHE["nc"]
    nc = bacc.Bacc("TRN2", target_bir_lowering=False, debug=False,
                   enable_asserts=False, num_devices=8)
    ins = {
        "x": nc.dram_tensor("x", [RC, H], F32, kind="ExternalInput").ap(),
        "freqs": nc.dram_tensor("freqs", [RC, 2, 32], F32, kind="ExternalInput").ap(),
        "wqkvT": nc.dram_tensor("wqkvT", [H, H3], F32, kind="ExternalInput").ap(),
        "woT": nc.dram_tensor("woT", [H, H], F32, kind="ExternalInput").ap(),
        "modwT": nc.dram_tensor("modwT", [H, 768], F32, kind="ExternalInput").ap(),
        "modb": nc.dram_tensor("modb", [128, 6], F32, kind="ExternalInput").ap(),
        "ada": nc.dram_tensor("ada", [128, 8], F32, kind="ExternalInput").ap(),
        "lnw": nc.dram_tensor("lnw", [1, H], F32, kind="ExternalInput").ap(),
        "qnw": nc.dram_tensor("qnw", [1, HD], F32, kind="ExternalInput").ap(),
        "knw": nc.dram_tensor("knw", [1, HD], F32, kind="ExternalInput").ap(),
    }
    out = nc.dram_tensor("out", [RC, H], F32, kind="ExternalOutput").ap()
    with tile.TileContext(nc) as tc:
        _emit(tc, ins, out)
    nc.compile()
    _CACHE["nc"] = nc
    return nc


def _shard(inputs):
    x = np.ascontiguousarray(np.asarray(inputs["x"], np.float32).reshape(R, H))
    ada = np.asarray(inputs["ada_cond"], np.float32)
    freqs = np.asarray(inputs["freqs"], np.float32)
    wqkvT = np.ascontiguousarray(np.asarray(inputs["w_qkv"], np.float32).T)
    woT = np.ascontiguousarray(np.asarray(inputs["w_o"], np.float32).T)
    modw = np.asarray(inputs["mod_w"], np.float32)
    modb = np.asarray(inputs["mod_b"], np.float32)
    lnw = np.asarray(inputs["ln_w"], np.float32).reshape(1, H)
    qnw = np.asarray(inputs["qn_w"], np.float32).reshape(1, HD)
    knw = np.asarray(inputs["kn_w"], np.float32).reshape(1, HD)

    in_maps = []
    for c in range(8):
        b, chunk = c // 4, c % 4
        s0 = (RC * c) % S
        in_maps.append({
            "x": np.ascontiguousarray(x[RC * c:RC * (c + 1)]),
            "freqs": np.ascontiguousarray(freqs[s0:s0 + RC].transpose(0, 2, 1)),
            "wqkvT": wqkvT,
            "woT": woT,
            "modwT": np.ascontiguousarray(modw[768 * chunk:768 * (chunk + 1)].T),
            "modb": np.ascontiguousarray(
                modb[768 * chunk:768 * (chunk + 1)].reshape(6, 128).T),
            "ada": np.ascontiguousarray(ada[b].reshape(8, 128).T),
            "lnw": lnw, "qnw": qnw, "knw": knw,
        })
    return in_maps


def _run(inputs, **kw):
    nc = _build()
    res = run_bass_kernel_spmd(nc, _shard(inputs), core_ids=list(range(8)), **kw)
    out = np.concatenate([res.results[c]["out"] for c in range(8)], axis=0)
    return out.reshape(B, S, H), res


def kernel(**inputs) -> np.ndarray:
    out, _ = _run(inputs)
    return out


# revision 26
# speedup vs baseline: 153.5757x; 153.5757x over previous
"""Trainium2 Bass kernel for nn_Attention_54855322304634 (8 NeuronCores).

Strategy (sequence-parallel + per-batch K/V AllGather):
- rows = flattened (B*S) = 4096; core c owns rows [512c, 512c+512); batch = c//4.
- AdaLN modulation: each core computes a 768-wide chunk (index c%4) of
  mod = silu(ada_b) @ mod_w.T + mod_b for ITS batch; AllGather over the
  4 cores of the batch reassembles the full [3072] modulation vector.
- LayerNorm + modulation in [rows, H] layout, PE-transpose to h_T [H, rows].
- QKV projection with host-transposed w_qkv (f32r matmuls, full PE rate).
- Per-head QK LayerNorm + RoPE in [rows, heads, hd]; q folded with 1/sqrt(hd);
  PE-transpose q,k to [2*64 pairdim, headpair, rows] bf16.
- AllGather (per-batch groups) of bf16 k_T and v.
- Attention per head: scores_T = K_hT.T @ q_hT (softmax axis on partitions),
  exp with NO max subtraction (max |score| ~18 for this problem's data),
  PV matmul with ones-augmented V so the denominator falls out of the same
  accumulation; normalize via reciprocal + stride-0 DMA partition broadcast.
- o-proj with host-transposed w_o (f32r), gate multiply, write own row-slice.
"""

import sys

if "/opt/trn_rl_repo" not in sys.path:
    sys.path.insert(0, "/opt/trn_rl_repo")

import numpy as np

import concourse.bass as bass
import concourse.tile as tile
from concourse import bacc, mybir
from concourse.bass_utils import run_bass_kernel_spmd
from concourse.masks import make_identity

F32 = mybir.dt.float32
F32R = mybir.dt.float32r
BF16 = mybir.dt.bfloat16
AX = mybir.AxisListType
OP = mybir.AluOpType
ACT = mybir.ActivationFunctionType

NH, HD, H, B, S, A = 16, 64, 1024, 2, 2048, 1024
EPS = 1e-5
R = B * S                # 4096 rows total
RC = R // 8              # 512 rows per core
NRT = RC // 128          # 4 row tiles per core
KC = S // 128            # 16 key chunks per head
H3 = 3 * H

_KF = 128 * 4 * RC       # flat bf16 elems of k_T region per AG half
_VF = RC * 8 * 65        # flat bf16 elems of v region per AG half (ones interleaved)


def _bc(ap, p):
    """Stride-0 partition broadcast to [p, ...].

    If dim0 has size 1 it is replaced; otherwise a new stride-0 partition
    dim is prepended (AP dims are [stride, size] pairs).
    """
    dims = list(ap.ap)
    if dims[0][1] == 1:
        dims = dims[1:]
    return bass.AP(tensor=ap.tensor, offset=ap.offset, ap=[[0, p]] + dims)


def _r(ap):
    return ap.bitcast(F32R)


def _emit(tc, ins, out, upto="D"):
    nc = tc.nc
    x_in, freqs_in, wqkvT_in, woT_in, modwT_in, modb_in, ada_in, lnw_in, qnw_in, knw_in = (
        ins["x"], ins["freqs"], ins["wqkvT"], ins["woT"], ins["modwT"],
        ins["modb"], ins["ada"], ins["lnw"], ins["qnw"], ins["knw"],
    )

    const = tc.alloc_tile_pool(name="const", bufs=1)
    pers = tc.alloc_tile_pool(name="pers", bufs=1)
    dram = tc.alloc_tile_pool(name="dram", bufs=1, space="DRAM")

    # ---------------- constants ----------------
    ident = const.tile([128, 128], F32)
    make_identity(nc, ident)
    eps128 = const.tile([128, 1], F32)
    nc.vector.memset(eps128, EPS)

    lnw_rep = const.tile([128, H], F32)
    nc.sync.dma_start(out=lnw_rep, in_=_bc(lnw_in, 128))
    qn_eff = const.tile([128, HD], F32)
    nc.sync.dma_start(out=qn_eff, in_=_bc(qnw_in, 128))
    nc.vector.tensor_scalar_mul(qn_eff, qn_eff, 0.125)  # fold 1/sqrt(hd) into q
    kn_rep = const.tile([128, HD], F32)
    nc.sync.dma_start(out=kn_rep, in_=_bc(knw_in, 128))

    f0t, f1t = [], []
    for rt in range(NRT):
        f0 = const.tile([128, 32], F32, tag=f"f0_{rt}")
        f1 = const.tile([128, 32], F32, tag=f"f1_{rt}")
        nc.sync.dma_start(out=f0, in_=freqs_in[rt * 128:(rt + 1) * 128, 0, :])
        nc.sync.dma_start(out=f1, in_=freqs_in[rt * 128:(rt + 1) * 128, 1, :])
        f0t.append(f0)
        f1t.append(f1)

    # ---------------- modulation chunk + AllGather ----------------
    ag1_in = dram.tile([1, 768], F32)
    ag1_out = dram.tile([4, 768], F32)

    with tc.tile_pool(name="modp", bufs=1) as modp, \
         tc.tile_pool(name="modpsum", bufs=2, space="PSUM") as modpsum:
        modwT_sb = modp.tile([128, 8, 768], F32R)
        nc.sync.dma_start(out=modwT_sb, in_=modwT_in.rearrange("(kt p) m -> p kt m", p=128).bitcast(F32R))
        modb_sb = modp.tile([128, 6], F32)
        nc.sync.dma_start(out=modb_sb, in_=modb_in)
        ada_sb = modp.tile([128, 8], F32)
        nc.sync.dma_start(out=ada_sb, in_=ada_in)
        # fp32r matmul needs even innermost free sizes -> pad N=1 to N=2
        silu_sb = modp.tile([128, 8, 2], F32R)
        nc.vector.memset(silu_sb[:].bitcast(F32), 0.0)
        nc.scalar.activation(out=silu_sb[:, :, 0], in_=ada_sb, func=ACT.Sigmoid)
        nc.vector.tensor_tensor(
            out=silu_sb[:, :, 0], in0=silu_sb[:, :, 0], in1=ada_sb, op=OP.mult)

        mod_sb = modp.tile([128, 6], F32)
        for t in range(6):
            ps = modpsum.tile([128, 2], F32, tag="modps")
            for kt in range(8):
                nc.tensor.matmul(
                    ps, modwT_sb[:, kt, t * 128:(t + 1) * 128],
                    silu_sb[:, kt, :], start=(kt == 0), stop=(kt == 7),
                )
            nc.vector.tensor_tensor(
                out=mod_sb[:, t:t + 1], in0=ps[:, 0:1], in1=modb_sb[:, t:t + 1], op=OP.add)
        mps = modpsum.tile([6, 128], F32, tag="modtp")
        nc.tensor.transpose(mps, mod_sb, ident)
        modT = modp.tile([6, 128], F32)
        nc.vector.tensor_copy(out=modT, in_=mps)
        nc.sync.dma_start(
            out=ag1_in[0, :].rearrange("(t p) -> t p", p=128), in_=modT)

    nc.gpsimd.collective_compute(
        "AllGather", OP.bypass,
        ins=[ag1_in[:].opt()], outs=[ag1_out[:].opt()],
        replica_groups=[[0, 1, 2, 3], [4, 5, 6, 7]],
    )

    # vec3_rep[p, :] = full [3072] modulation vector, replicated on partitions.
    # scaleT/shiftT: column j of transpose(replicated matrix) is the vector.
    scaleT = pers.tile([128, 8], F32)
    shiftT = pers.tile([128, 8], F32)
    gate_v = pers.tile([128, H], F32, name="gate_v")
    with tc.tile_pool(name="modstage", bufs=1) as modstage, \
         tc.tile_pool(name="modpsA", bufs=2, space="PSUM") as modpsA:
        vec3_rep = modstage.tile([128, 3072], F32)
        nc.sync.dma_start(out=vec3_rep, in_=_bc(ag1_out[:].rearrange("a b -> (a b)"), 128))
        scale1p = modstage.tile([128, H], F32)
        nc.vector.tensor_scalar_add(scale1p, vec3_rep[:, 0:H], 1.0)
        nc.vector.tensor_tensor(out=scale1p, in0=scale1p, in1=lnw_rep, op=OP.mult)
        shift_v = vec3_rep[:, H:2 * H]
        nc.vector.tensor_copy(out=gate_v, in_=vec3_rep[:, 2 * H:3 * H])
        for kt in range(8):
            pt = modpsA.tile([128, 128], F32, tag="tpm")
            nc.tensor.transpose(pt, scale1p[:, kt * 128:(kt + 1) * 128], ident)
            nc.vector.tensor_copy(out=scaleT[:, kt:kt + 1], in_=pt[:, 0:1])
            pt2 = modpsA.tile([128, 128], F32, tag="tpm")
            nc.tensor.transpose(pt2, shift_v[:, kt * 128:(kt + 1) * 128], ident)
            nc.vector.tensor_copy(out=shiftT[:, kt:kt + 1], in_=pt2[:, 0:1])

    if upto == "mod":
        dram.release(); pers.release(); const.release()
        return

    # ---------------- phase A: LN + transpose to h_T (modulation fused into
    # the PSUM->SBUF copy as per-partition scale/bias in the transposed domain)
    pool1 = tc.alloc_tile_pool(name="qk_pers", bufs=1)
    hT = pool1.tile([128, 8, RC], F32R)   # [H-part, H-chunk, rows]

    with tc.tile_pool(name="xin", bufs=1) as xin, \
         tc.tile_pool(name="xa", bufs=2) as xa, \
         tc.tile_pool(name="stats", bufs=2) as stats, \
         tc.tile_pool(name="tpsum", bufs=4, space="PSUM") as tpsum:
        xall = xin.tile([128, NRT, H], F32, tag="xall")
        nc.sync.dma_start(out=xall, in_=x_in.rearrange("(rt p) h -> p rt h", p=128))
        for rt in range(NRT):
            xt = xall[:, rt, :]
            st = stats.tile([128, 2, 6], F32, tag="bnst")
            nc.vector.bn_stats(out=st[:, 0, :], in_=xt[:, 0:512])
            nc.vector.bn_stats(out=st[:, 1, :], in_=xt[:, 512:1024])
            mv = stats.tile([128, 2], F32, tag="bnmv")
            nc.vector.bn_aggr(out=mv, in_=st)
            rstd = stats.tile([128, 1], F32, tag="rstd")
            nc.scalar.activation(out=rstd, in_=mv[:, 1:2], func=ACT.Sqrt, bias=eps128)
            nc.vector.reciprocal(rstd, rstd)
            hmod = xa.tile([128, H], F32, tag="hmod")
            nc.vector.tensor_scalar(
                out=hmod, in0=xt, scalar1=mv[:, 0:1], scalar2=rstd,
                op0=OP.subtract, op1=OP.mult)
            for kt in range(8):
                pt = tpsum.tile([128, 128], F32, tag="tp")
                nc.tensor.transpose(pt, hmod[:, kt * 128:(kt + 1) * 128], ident)
                nc.scalar.activation(
                    out=hT[:, kt, rt * 128:(rt + 1) * 128], in_=pt,
                    func=ACT.Identity, scale=scaleT[:, kt:kt + 1],
                    bias=shiftT[:, kt:kt + 1])

    if upto == "A":
        pool1.release(); dram.release(); pers.release(); const.release()
        return

    # ---------------- phase B: QKV + qk-LN + RoPE + transposes ----------------
    qT = pool1.tile([128, 8, RC], BF16)
    ag2_in = [dram.tile([1, _KF + _VF], BF16, name=f"ag2i{i}") for i in range(2)]
    ag2_out = [dram.tile([4, _KF + _VF], BF16, name=f"ag2o{i}") for i in range(2)]
    k_regions = [a[0, 0:_KF].rearrange("(p h r) -> p h r", p=128, h=4) for a in ag2_in]
    v_regions = [a[0, _KF:].rearrange("(r c) -> r c", r=RC) for a in ag2_in]  # [RC, 520]

    # rope factor tiles with qn/kn (and 1/sqrt(hd) for q) folded in:
    # re = t0*g00 - t1*g11 ; im = t1*g10 + t0*g01
    gfac = {}
    for rt in range(NRT):
        for is_q in (True, False):
            w_rep = qn_eff if is_q else kn_rep
            we, wo = w_rep[:, 0::2], w_rep[:, 1::2]
            g = [const.tile([128, 32], F32, tag=f"g{rt}{is_q}{i}", name=f"g{rt}{is_q}{i}")
                 for i in range(4)]
            nc.vector.tensor_tensor(out=g[0], in0=f0t[rt], in1=we, op=OP.mult)
            nc.vector.tensor_tensor(out=g[1], in0=f1t[rt], in1=wo, op=OP.mult)
            nc.vector.tensor_tensor(out=g[2], in0=f0t[rt], in1=wo, op=OP.mult)
            nc.vector.tensor_tensor(out=g[3], in0=f1t[rt], in1=we, op=OP.mult)
            gfac[(rt, is_q)] = g

    def qk_process(nc_, ps, rt, is_q, dst, hp0):
        """ps: PSUM [128, 512] f32 = 8 heads of q or k for row tile rt."""
        t3 = ps[:].rearrange("p (h d) -> p h d", h=8)
        s1 = stats2.tile([128, 8], F32, tag="s1")
        s2 = stats2.tile([128, 8], F32, tag="s2")
        sq = work.tile([128, 512], F32, tag="sq")
        nc_.vector.tensor_reduce(out=s1, in_=t3, axis=AX.X, op=OP.add)
        nc_.scalar.activation(out=sq, in_=ps[:], func=ACT.Square)
        nc_.vector.tensor_reduce(out=s2, in_=sq[:].rearrange("p (h d) -> p h d", h=8),
                                 axis=AX.X, op=OP.add)
        nc_.vector.tensor_scalar_mul(s1, s1, 1.0 / HD)           # mean
        nc_.vector.tensor_scalar_mul(s2, s2, 1.0 / HD)           # mean(x^2)
        m2 = stats2.tile([128, 8], F32, tag="m2")
        nc_.vector.tensor_tensor(out=m2, in0=s1, in1=s1, op=OP.mult)
        nc_.vector.tensor_tensor(out=s2, in0=s2, in1=m2, op=OP.subtract)  # var
        nc_.scalar.activation(out=s2, in_=s2, func=ACT.Sqrt, bias=eps128)
        nc_.vector.reciprocal(s2, s2)                            # rstd
        y = work.tile([128, 512], F32, tag="y")
        y3 = y[:].rearrange("p (h d) -> p h d", h=8)
        nc_.vector.tensor_tensor(out=y3, in0=t3, in1=s1[:, :, None].to_broadcast((128, 8, 64)), op=OP.subtract)
        nc_.vector.tensor_tensor(out=y3, in0=y3, in1=s2[:, :, None].to_broadcast((128, 8, 64)), op=OP.mult)
        # RoPE (qn/kn folded into g factors)
        g = gfac[(rt, is_q)]
        gb = [gi[:, None, :].to_broadcast((128, 8, 32)) for gi in g]
        y4 = y[:].rearrange("p (h d2 two) -> p h d2 two", h=8, two=2)
        ro = work.tile([128, 512], F32, tag="ro")
        ro4 = ro[:].rearrange("p (h d2 two) -> p h d2 two", h=8, two=2)
        tm = work.tile([128, 256], F32, tag="tm")
        tm3 = tm[:].rearrange("p (h d2) -> p h d2", h=8)
        nc_.vector.tensor_tensor(out=ro4[:, :, :, 0], in0=y4[:, :, :, 0], in1=gb[0], op=OP.mult)
        nc_.vector.tensor_tensor(out=tm3, in0=y4[:, :, :, 1], in1=gb[1], op=OP.mult)
        nc_.vector.tensor_tensor(out=ro4[:, :, :, 0], in0=ro4[:, :, :, 0], in1=tm3, op=OP.subtract)
        nc_.vector.tensor_tensor(out=ro4[:, :, :, 1], in0=y4[:, :, :, 1], in1=gb[2], op=OP.mult)
        nc_.vector.tensor_tensor(out=tm3, in0=y4[:, :, :, 0], in1=gb[3], op=OP.mult)
        nc_.vector.tensor_tensor(out=ro4[:, :, :, 1], in0=ro4[:, :, :, 1], in1=tm3, op=OP.add)
        # transpose 4 head-pairs -> dst[:, hp0+pair, rt*128:...]
        for pr in range(4):
            pt = tpsum2.tile([128, 128], F32, tag="tp2")
            nc_.tensor.transpose(pt, ro[:, pr * 128:(pr + 1) * 128], ident)
            nc_.scalar.copy(
                out=dst[:, hp0 + pr, rt * 128:(rt + 1) * 128], in_=pt)

    # ---------------- merged QKV + AllGather + attention flow ----------------
    pool2 = tc.alloc_tile_pool(name="att_pers", bufs=1)
    woT_sb = pool2.tile([128, 8, H], F32R)
    nc.sync.dma_start(out=woT_sb, in_=woT_in.rearrange("(kt p) m -> p kt m", p=128).bitcast(F32R))
    KT = pool2.tile([128, 8, S], BF16)
    oT = pool2.tile([128, 8, RC], F32R)
    vsb = pool2.tile([128, KC, NH * 65], BF16)

    with tc.tile_pool(name="wq", bufs=1) as wq, \
         tc.tile_pool(name="work", bufs=2) as work, \
         tc.tile_pool(name="stats2", bufs=3) as stats2, \
         tc.tile_pool(name="kstage", bufs=1) as kstage, \
         tc.tile_pool(name="esb", bufs=4) as esb, \
         tc.tile_pool(name="rec", bufs=2) as recp, \
         tc.tile_pool(name="qkpsum", bufs=1, space="PSUM") as qkpsum, \
         tc.tile_pool(name="tpsum2", bufs=1, space="PSUM") as tpsum2, \
         tc.tile_pool(name="spsum", bufs=2, space="PSUM") as spsum, \
         tc.tile_pool(name="opsum", bufs=2, space="PSUM") as opsum:
        wq_src = wqkvT_in.rearrange("(kt p) n -> p kt n", p=128).bitcast(F32R)

        def qkv_chunk(nch, half):
            wt = wq.tile([128, 8, 512], F32R, tag="wt", name=f"wt{nch}")
            nc.sync.dma_start(out=wt, in_=wq_src[:, :, nch * 512:(nch + 1) * 512])
            kstg = None
            if nch in (2, 3):
                kstg = kstage.tile([128, 4, 512], BF16, tag="kstg", name=f"kstg{nch}")
            for rt in range(NRT):
                ps = qkpsum.tile([128, 512], F32, tag="qkps", name=f"qkps{nch}_{rt}")
                for kt in range(8):
                    nc.tensor.matmul(
                        ps, hT[:, kt, rt * 128:(rt + 1) * 128], wt[:, kt, :],
                        start=(kt == 0), stop=(kt == 7))
                if nch >= 4:      # v chunk: cast + interleave ones + ship
                    vt = work.tile([128, 8, 65], BF16, tag="vt", name=f"vt{nch}_{rt}")
                    nc.vector.tensor_copy(
                        out=vt[:, :, 0:64],
                        in_=ps[:].rearrange("p (h d) -> p h d", h=8))
                    nc.vector.memset(vt[:, :, 64:65], 1.0)
                    nc.sync.dma_start(
                        out=v_regions[half][rt * 128:(rt + 1) * 128, :], in_=vt)
                elif nch in (0, 1):   # q
                    qk_process(nc, ps, rt, True, qT, nch * 4)
                else:                 # k -> staging then AG buffer
                    qk_process(nc, ps, rt, False, kstg, 0)
            if nch in (2, 3):
                nc.sync.dma_start(out=k_regions[half][:], in_=kstg)

        def emit_ag(half):
            if "noag" in upto:
                return
            nc.gpsimd.collective_compute(
                "AllGather", OP.bypass,
                ins=[ag2_in[half][:].opt()], outs=[ag2_out[half][:].opt()],
                replica_groups=[[0, 1, 2, 3], [4, 5, 6, 7]],
            )

        def half_loads(half):
            for r in range(4):
                nc.sync.dma_start(
                    out=KT[:, 4 * half:4 * (half + 1), r * RC:(r + 1) * RC],
                    in_=ag2_out[half][r, 0:_KF].rearrange(
                        "(p h q) -> p h q", p=128, h=4))
                nc.sync.dma_start(
                    out=vsb[:, 4 * r:4 * (r + 1), 520 * half:520 * (half + 1)],
                    in_=ag2_out[half][r, _KF:].rearrange(
                        "(a p c) -> p a c", a=4, p=128))

        def emit_head(h):
            hp, lo = h // 2, (h % 2) * 64
            q_h = qT[lo:lo + 64, hp, :]
            po = opsum.tile([128, 512], F32, tag="ops", name=f"po{h}")
            for kg in range(KC // 2):
                ps = spsum.tile([128, 2, 512], F32, tag="sps", name=f"sps{h}_{kg}")
                for j in range(2):
                    kc = 2 * kg + j
                    nc.tensor.matmul(
                        ps[:, j, :], KT[lo:lo + 64, hp, kc * 128:(kc + 1) * 128],
                        q_h, start=True, stop=True)
                et = esb.tile([128, 2, 512], BF16, tag="et", name=f"et{h}_{kg}")
                nc.scalar.activation(out=et, in_=ps, func=ACT.Exp)
                for j in range(2):
                    kc = 2 * kg + j
                    vcol = (h // 8) * 520 + (h % 8) * 65
                    nc.tensor.matmul(
                        po[0:65, :], vsb[:, kc, vcol:vcol + 65], et[:, j, :],
                        start=(kc == 0), stop=(kc == KC - 1))
            rec = recp.tile([128, 512], F32, tag="rec", name=f"rec{h}")
            nc.vector.reciprocal(rec[64:65, :], po[64:65, :])
            dden = dram.tile([1, 512], F32, tag=f"dden{h % 4}", name=f"dden{h}")
            nc.sync.dma_start(out=dden, in_=rec[64:65, :])
            recb = recp.tile([64, 512], F32, tag="recb", name=f"recb{h}")
            nc.sync.dma_start(out=recb, in_=_bc(dden[:], 64))
            o_n = recp.tile([64, 512], F32R, tag="o_n", name=f"o_n{h}")
            nc.vector.tensor_tensor(out=o_n, in0=po[0:64, :], in1=recb, op=OP.mult)
            nc.sync.dma_start(out=oT[lo:lo + 64, hp, :], in_=o_n)

        qkv_chunk(2, 0)       # k heads 0-7
        qkv_chunk(4, 0)       # v heads 0-7
        emit_ag(0)
        qkv_chunk(0, 0)       # q heads 0-7
        half_loads(0)
        qkv_chunk(3, 1)       # k heads 8-15
        for h in range(0, 4):
            emit_head(h)
        qkv_chunk(5, 1)       # v heads 8-15
        for h in range(4, 8):
            emit_head(h)
        emit_ag(1)
        qkv_chunk(1, 1)       # q heads 8-15
        half_loads(1)
        for h in range(8, NH):
            emit_head(h)

    if upto.startswith("C"):
        pool2.release(); pool1.release(); dram.release(); pers.release(); const.release()
        return

    # ---------------- phase D: o-proj + gate ----------------
    with tc.tile_pool(name="outp", bufs=2) as outp, \
         tc.tile_pool(name="prpsum", bufs=2, space="PSUM") as prpsum:
        for rt in range(NRT):
            for nch in range(2):
                ps = prpsum.tile([128, 512], F32, tag="prps")
                for kt in range(8):
                    nc.tensor.matmul(
                        ps, oT[:, kt, rt * 128:(rt + 1) * 128],
                        woT_sb[:, kt, nch * 512:(nch + 1) * 512],
                        start=(kt == 0), stop=(kt == 7))
                ot = outp.tile([128, 512], F32, tag="ot")
                nc.vector.tensor_tensor(
                    out=ot, in0=ps, in1=gate_v[:, nch * 512:(nch + 1) * 512],
                    op=OP.mult)
                nc.sync.dma_start(
                    out=out[rt * 128:(rt + 1) * 128, nch * 512:(nch + 1) * 512],
                    in_=ot)

    pool2.release()
    pool1.release()
    dram.release()
    pers.release()
    const.release()


_CACHE = {}


def _build(upto="D"):
    if ("nc", upto) in _CACHE:
        return _CACHE[("nc", upto)]
    nc = bacc.Bacc("TRN2", target_bir_lowering=False, debug=False,
                   enable_asserts=False, num_devices=8)
    ins = {
        "x": nc.dram_tensor("x", [RC, H], F32, kind="ExternalInput").ap(),
        "freqs": nc.dram_tensor("freqs", [RC, 2, 32], F32, kind="ExternalInput").ap(),
        "wqkvT": nc.dram_tensor("wqkvT", [H, H3], F32, kind="ExternalInput").ap(),
        "woT": nc.dram_tensor("woT", [H, H], F32, kind="ExternalInput").ap(),
        "modwT": nc.dram_tensor("modwT", [H, 768], F32, kind="ExternalInput").ap(),
        "modb": nc.dram_tensor("modb", [128, 6], F32, kind="ExternalInput").ap(),
        "ada": nc.dram_tensor("ada", [128, 8], F32, kind="ExternalInput").ap(),
        "lnw": nc.dram_tensor("lnw", [1, H], F32, kind="ExternalInput").ap(),
        "qnw": nc.dram_tensor("qnw", [1, HD], F32, kind="ExternalInput").ap(),
        "knw": nc.dram_tensor("knw", [1, HD], F32, kind="ExternalInput").ap(),
    }
    out = nc.dram_tensor("out", [RC, H], F32, kind="ExternalOutput").ap()
    with tile.TileContext(nc) as tc:
        _emit(tc, ins, out, upto=upto)
    nc.compile()
    _CACHE[("nc", upto)] = nc
    return nc


def _shard(inputs):
    x = np.ascontiguousarray(np.asarray(inputs["x"], np.float32).reshape(R, H))
    ada = np.asarray(inputs["ada_cond"], np.float32)
    freqs = np.asarray(inputs["freqs"], np.float32)
    wqkvT = np.ascontiguousarray(np.asarray(inputs["w_qkv"], np.float32).T)
    woT = np.ascontiguousarray(np.asarray(inputs["w_o"], np.float32).T)
    modw = np.asarray(inputs["mod_w"], np.float32)
    modb = np.asarray(inputs["mod_b"], np.float32)
    lnw = np.asarray(inputs["ln_w"], np.float32).reshape(1, H)
    qnw = np.asarray(inputs["qn_w"], np.float32).reshape(1, HD)
    knw = np.asarray(inputs["kn_w"], np.float32).reshape(1, HD)

    in_maps = []
    for c in range(8):
        b, chunk = c // 4, c % 4
        s0 = (RC * c) % S
        in_maps.append({
            "x": np.ascontiguousarray(x[RC * c:RC * (c + 1)]),
            "freqs": np.ascontiguousarray(freqs[s0:s0 + RC].transpose(0, 2, 1)),
            "wqkvT": wqkvT,
            "woT": woT,
            "modwT": np.ascontiguousarray(modw[768 * chunk:768 * (chunk + 1)].T),
            "modb": np.ascontiguousarray(
                modb[768 * chunk:768 * (chunk + 1)].reshape(6, 128).T),
            "ada": np.ascontiguousarray(ada[b].reshape(8, 128).T),
            "lnw": lnw, "qnw": qnw, "knw": knw,
        })
    return in_maps


def _run(inputs, **kw):
    nc = _build()
    res = run_bass_kernel_spmd(nc, _shard(inputs), core_ids=list(range(8)), **kw)
    out = np.concatenate([res.results[c]["out"] for c in range(8)], axis=0)
    return out.reshape(B, S, H), res


def kernel(**inputs) -> np.ndarray:
    out, _ = _run(inputs)
    return out
